# revision 12
# baseline (speedup 1.0000x reference)
"""CRF Viterbi decode (tf crf_decode semantics) on 8 Trainium2 cores.

Strategy (data-parallel, 16 batches/core, exact fp32 vs the jax reference):
  Forward: per step t, per (group of 8 batches, j-tile, batch) one fused
    custom DVE op computes newstate[j] = max_i(A[i,j] + s[i]) + x_t[j];
    the prev state is broadcast across partitions by K=1 PE matmuls into
    PSUM.  Only state VALUES are stored (to DRAM); no backpointers.
  Backward: per step tau (descending), the one-hot of tag_tau gathers
    column A[:, tag] via a PE matmul; a custom DVE op with a reversed
    max-scan recovers the first-argmax backpointer exactly.
  Host: inputs past each sequence length are masked to -1e30; backward
    gating (vmask) reproduces the reference freeze/identity semantics.
"""

import numpy as np
from contextlib import ExitStack

B, T, C = 128, 1024, 256
NCORES = 8
BLOC = B // NCORES  # 16
G = 8               # batches per forward group (2 groups/core)
BLKT = 32           # time-block size

_CACHE = {}


# ----------------------------------------------------------------------------
# custom DVE ops
# ----------------------------------------------------------------------------
def register_ops():
    from concourse.dve_ops import (
        DveOp, OPS, CUSTOM_DVE_SPECS, _SUB_OPCODE_FOR_NAME, _CUSTOM_DVE_ROW_BASE,
    )
    from concourse.dve_spec import (
        Spec, Src0, Src1, C0, C1, AluOp, Idx, eq, select, scan,
    )

    if "CRF_MAXPLUS" in CUSTOM_DVE_SPECS:
        by_name = {op.name: op for op in OPS}
        return by_name["CRF_MAXPLUS"], by_name["CRF_ARGMAX_REV"]

    def ref_maxplus(in0, in1, c0, c1, imm2):
        out = (in0 + in1) + c0
        return out, out.max(axis=-1, keepdims=True)

    MAXPLUS = DveOp(
        "CRF_MAXPLUS",
        Spec(body=(Src0 + Src1) + C0, accum=AluOp.MAX, reference=ref_maxplus),
        subdim=False, uops_sha={},
    )

    def ref_argmax_rev(in0, in1, c0, c1, imm2):
        v = in0 + in1
        r = np.maximum.accumulate(v, axis=-1)
        idx = np.arange(v.shape[-1], dtype=np.float32)[None, :]
        out = np.where(v == r, idx, c1)
        return out, out.max(axis=-1, keepdims=True)

    _v = Src0 + Src1
    ARGMAXR = DveOp(
        "CRF_ARGMAX_REV",
        Spec(body=select(eq(_v, scan(AluOp.MAX, _v)), Idx, C1),
             accum=AluOp.MAX, reference=ref_argmax_rev),
        subdim=False, uops_sha={},
    )

    import re
    for op in (MAXPLUS, ARGMAXR):
        OPS.append(op)
        CUSTOM_DVE_SPECS[op.name] = op.spec
        _SUB_OPCODE_FOR_NAME[op.name] = _CUSTOM_DVE_ROW_BASE + len(OPS) - 1
        for ver in ("v3", "v4"):
            try:
                op.compile(ver)
            except ValueError as e:
                m = re.search(r'"([0-9a-f]{16})"', str(e))
                assert m, f"no sha in: {e}"
                op.uops_sha[ver] = m.group(1)
                op.compile(ver)
    return MAXPLUS, ARGMAXR


# ----------------------------------------------------------------------------
# kernel builder
# ----------------------------------------------------------------------------
def build_nc(t_total=T):
    import concourse.bass as bass
    import concourse.bacc as bacc
    import concourse.mybir as mybir
    from concourse import tile, masks

    MAXPLUS, ARGMAXR = register_ops()

    f32 = mybir.dt.float32
    i32 = mybir.dt.int32
    ds = bass.ds
    AO = mybir.AluOpType
    nblk = t_total // BLKT

    nc = bacc.Bacc("TRN2", target_bir_lowering=False, debug=False,
                   num_devices=NCORES)

    x_d = nc.dram_tensor("x", [BLOC, t_total, C], f32, kind="ExternalInput")
    A_d = nc.dram_tensor("trans", [C, C], f32, kind="ExternalInput")
    vm_d = nc.dram_tensor("vmask", [BLOC, t_total + 32], f32,
                          kind="ExternalInput")
    lb1_d = nc.dram_tensor("lb1", [1, BLOC], i32, kind="ExternalInput")
    out_d = nc.dram_tensor("out", [BLOC, t_total, C], f32,
                           kind="ExternalOutput")
    xT_d = nc.dram_tensor("xT", [2, 128, BLOC, t_total], f32)
    st2_d = nc.dram_tensor("states2", [BLOC, t_total, C], f32)

    iota_rev_np = np.tile((C - 1 - np.arange(C)).astype(np.float32), (BLOC, 1))
    iota_rev_d = nc.inline_tensor(iota_rev_np, name="iota_rev")
    eb_np = np.zeros((G, G * 128), dtype=np.float32)
    for _b in range(G):
        eb_np[_b, _b * 128:(_b + 1) * 128] = 1.0
    eb_d = nc.inline_tensor(eb_np, name="eb_sel")

    with tile.TileContext(nc) as tc, ExitStack() as ctx:
        # ------------------------- constant tiles --------------------------
        cpool = ctx.enter_context(tc.tile_pool(name="consts", bufs=1))
        ident = cpool.tile([128, 128], f32)
        masks.make_identity(nc, ident[:])
        ones_row = cpool.tile([1, 128], f32)
        nc.vector.memset(ones_row[:], 1.0)
        iota_rev = cpool.tile([BLOC, C], f32)
        nc.sync.dma_start(iota_rev[:], iota_rev_d.ap())
        zeros_bc = cpool.tile([BLOC, C], f32)
        nc.vector.memset(zeros_bc[:], 0.0)
        AT = [cpool.tile([128, C], f32, tag=f"AT{jt}", name=f"AT{jt}") for jt in range(2)]
        vm_sb = cpool.tile([BLOC, t_total + 32], f32)
        nc.sync.dma_start(vm_sb[:], vm_d.ap())
        lb1_sb = cpool.tile([1, BLOC], i32)
        nc.sync.dma_start(lb1_sb[:], lb1_d.ap())
        eb_sel = cpool.tile([G, G * 128], f32)
        nc.sync.dma_start(eb_sel[:], eb_d.ap())

        # ---------------- build AT (A transposed) and xT --------------------
        with tc.tile_pool(name="prep", bufs=3) as ppool, \
             tc.tile_pool(name="prep_ps", bufs=4, space="PSUM") as pspool:
            for it in range(2):
                a_sb = ppool.tile([128, C], f32, tag="a_sb")
                nc.sync.dma_start(a_sb[:], A_d.ap()[it * 128:(it + 1) * 128, :])
                for jt in range(2):
                    pt = pspool.tile([128, 128], f32, tag="pt")
                    nc.tensor.transpose(pt[:], a_sb[:, jt * 128:(jt + 1) * 128],
                                        ident[:])
                    nc.scalar.copy(AT[jt][:, it * 128:(it + 1) * 128], pt[:])

            # x[b, t, c] -> xT[jt][c', b, t]
            for b in range(BLOC):
                for t8 in range(t_total // 128):
                    xin = ppool.tile([128, C], f32, tag="xin")
                    nc.sync.dma_start(
                        xin[:], x_d.ap()[b, t8 * 128:(t8 + 1) * 128, :])
                    for jt in range(2):
                        pt2 = pspool.tile([128, 128], f32, tag="pt")
                        nc.tensor.transpose(
                            pt2[:], xin[:, jt * 128:(jt + 1) * 128], ident[:])
                        xo = ppool.tile([128, 128], f32, tag="xo")
                        nc.scalar.copy(xo[:], pt2[:])
                        nc.sync.dma_start(
                            xT_d.ap()[jt, :, b, t8 * 128:(t8 + 1) * 128],
                            xo[:])

        # ----------------------------- forward -----------------------------
        with tc.tile_pool(name="fwd", bufs=2) as fpool, \
             tc.tile_pool(name="fwd_ps", bufs=1, space="PSUM") as fps:

            scg = [fpool.tile([G, C], f32, tag=f"scg{g}", name=f"scg{g}") for g in range(2)]
            bcast = [fps.tile([128, G * C], f32, tag=f"bc{g}", name=f"bc{g}")
                     for g in range(2)]
            for g in range(2):
                nc.sync.dma_start(scg[g][:], x_d.ap()[g * G:(g + 1) * G, 0, :])

            def emit_fwd_step(u, rings, xstgs, ring2s=None):
                """one interleaved step for both groups at block slot u.
                scg holds s_{t-1}; after this, scg holds s_t."""
                for g in range(2):
                    for b in range(G):
                        nc.tensor.matmul(
                            out=bcast[g][:, b * C:(b + 1) * C],
                            lhsT=eb_sel[:, b * 128:(b + 1) * 128],
                            rhs=scg[g][:], start=True, stop=True)
                    for jt in range(2):
                        for b in range(G):
                            scr = fpool.tile([128, C], f32, tag="fscr")
                            nc.vector._custom_dve(
                                MAXPLUS, out=scr[:],
                                in0=AT[jt][:],
                                in1=bcast[g][:, b * C:(b + 1) * C],
                                s0=xstgs[g][jt][:, b, u:u + 1],
                                s1=0.0,
                                accum_out=rings[g][jt][:, u, b:b + 1])
                    for jt in range(2):
                        tp = bcast[g][0:G, jt * 128:(jt + 1) * 128]
                        nc.tensor.transpose(tp, rings[g][jt][:, u, :],
                                            ident[:])
                        nc.scalar.copy(scg[g][:, jt * 128:(jt + 1) * 128], tp)
                        nc.scalar.copy(
                            ring2s[g][:, u, jt * 128:(jt + 1) * 128], tp)

            def alloc_block_tiles():
                rings = [[fpool.tile([128, BLKT, G], f32, tag=f"ring{g}{jt}", name=f"ring{g}{jt}")
                          for jt in range(2)] for g in range(2)]
                xstgs = [[fpool.tile([128, G, BLKT], f32, tag=f"xstg{g}{jt}", name=f"xstg{g}{jt}")
                          for jt in range(2)] for g in range(2)]
                ring2s = [fpool.tile([G, BLKT, C], f32, tag=f"ring2{g}",
                                     name=f"ring2{g}") for g in range(2)]
                return rings, xstgs, ring2s

            def emit_xstg_load(xstgs, t0):
                for g in range(2):
                    for jt in range(2):
                        nc.sync.dma_start(
                            xstgs[g][jt][:],
                            xT_d.ap()[jt, :, g * G:(g + 1) * G, ds(t0, BLKT)])

            def emit_ring_store(ring2s, t0):
                for g in range(2):
                    nc.sync.dma_start(
                        st2_d.ap()[g * G:(g + 1) * G, ds(t0, BLKT), :],
                        ring2s[g][:])

            # block 0: slot 0 is the init state; update steps 1..BLKT-1
            rings, xstgs, ring2s = alloc_block_tiles()
            emit_xstg_load(xstgs, 0)
            for g in range(2):
                nc.sync.dma_start(ring2s[g][:, 0, :],
                                  x_d.ap()[g * G:(g + 1) * G, 0, :])
            for u in range(1, BLKT):
                emit_fwd_step(u, rings, xstgs, ring2s)
            emit_ring_store(ring2s, 0)

            with tc.For_i(1, nblk) as i:
                rings, xstgs, ring2s = alloc_block_tiles()
                emit_xstg_load(xstgs, i * BLKT)
                for u in range(BLKT):
                    emit_fwd_step(u, rings, xstgs, ring2s)
                emit_ring_store(ring2s, i * BLKT)

        # ----------------------------- backward ----------------------------
        with tc.tile_pool(name="bwd", bufs=2) as bpool, \
             tc.tile_pool(name="bwd1", bufs=1) as bpool1, \
             tc.tile_pool(name="bwd_ps", bufs=2, space="PSUM") as bps:

            # gather final states s_{len-1} per batch -> s_final [b, i]
            s_final = bpool1.tile([BLOC, C], f32, tag="s_final")
            for b in range(BLOC):
                tb = nc.sync.value_load(lb1_sb[0:1, b:b + 1])
                nc.sync.dma_start(s_final[b:b + 1, :],
                                  st2_d.ap()[b, ds(tb, 1), :])

            # c = reversed-coord argmax; c_init from s_final
            c_tile = bpool1.tile([BLOC, 1], f32, tag="c")
            bscr0 = bpool.tile([BLOC, C], f32, tag="bscr")
            nc.vector._custom_dve(
                ARGMAXR, out=bscr0[:], in0=s_final[:, ::-1], in1=zeros_bc[:],
                s0=0.0, s1=-1.0, accum_out=c_tile[:])

            lhsT = [bpool1.tile([128, BLOC], f32, tag=f"lhsT{jt}", name=f"lhsT{jt}")
                    for jt in range(2)]
            for jt in range(2):
                nc.vector.memset(lhsT[jt][:], 0.0)
            oh_scr = bpool1.tile([BLOC, C], f32, tag="oh_scr")

            nchunk = BLKT // 8  # 4 chunks of 8 time-slots
            with tc.For_i(0, nblk) as i:
                kk = nblk - 1 - i
                t0 = kk * BLKT
                stg2 = bpool.tile([BLOC, BLKT, C], f32, tag="stg2",
                                  name="stg2")
                nc.sync.dma_start(stg2[:], st2_d.ap()[:, ds(t0, BLKT), :])
                ochunks = [bpool1.tile([BLOC, 8, C], f32, tag=f"och{c8}", name=f"och{c8}")
                           for c8 in range(nchunk)]
                for u in range(BLKT):
                    m = BLKT - 1 - u
                    c8, tslot = m // 8, m % 8
                    colAT = bps.tile([BLOC, C], f32, tag="colAT")
                    for jt in range(2):
                        nc.tensor.matmul(out=colAT[:], lhsT=lhsT[jt][:],
                                         rhs=AT[jt][:], start=(jt == 0),
                                         stop=(jt == 1))
                    ktile = bpool.tile([BLOC, 1], f32, tag="ktile")
                    bscr = bpool.tile([BLOC, C], f32, tag="bscr")
                    nc.vector._custom_dve(
                        ARGMAXR, out=bscr[:],
                        in0=stg2[:, m, ::-1], in1=colAT[:, ::-1],
                        s0=0.0, s1=-1.0, accum_out=ktile[:])
                    # c += vmask[:, tau] * (k - c);  tau = t0 + m + 1
                    dtile = bpool.tile([BLOC, 1], f32, tag="dtile")
                    nc.vector.scalar_tensor_tensor(
                        out=dtile[:], in0=ktile[:], scalar=c_tile[:],
                        in1=vm_sb[:, ds(t0 + m + 1, 1)],
                        op0=AO.subtract, op1=AO.mult)
                    nc.vector.tensor_tensor(out=c_tile[:], in0=c_tile[:],
                                            in1=dtile[:], op=AO.add)
                    # one-hot output row tau-1 = t0 + m
                    nc.vector.tensor_scalar(
                        out=ochunks[c8][:, tslot, :], in0=iota_rev[:],
                        scalar1=c_tile[:], scalar2=None, op0=AO.is_equal)
                    # same one-hot into a base-0 scratch for the PE feed
                    nc.vector.tensor_scalar(
                        out=oh_scr[:], in0=iota_rev[:],
                        scalar1=c_tile[:], scalar2=None, op0=AO.is_equal)
                    # feed one-hot back as lhsT for the next colAT
                    for jt in range(2):
                        pso = bps.tile([128, BLOC], f32, tag="pso")
                        nc.tensor.transpose(
                            pso[:], oh_scr[:, jt * 128:(jt + 1) * 128],
                            ident[0:BLOC, 0:BLOC])
                        nc.scalar.copy(lhsT[jt][:], pso[:])
                    if tslot == 0:
                        nc.sync.dma_start(
                            out_d.ap()[:, ds(t0 + c8 * 8, 8), :],
                            ochunks[c8][:])

    nc.finalize()
    return nc


def _get_nc(t_total=T):
    key = ("nc", t_total)
    if key not in _CACHE:
        _CACHE[key] = build_nc(t_total)
    return _CACHE[key]


# ----------------------------------------------------------------------------
# host wrapper
# ----------------------------------------------------------------------------
def make_core_inputs(inputs, transitions, sequence_lengths, t_total=T):
    """Returns (in_maps, perm). Core c gets batches perm[c*BLOC:(c+1)*BLOC]."""
    lens = np.asarray(sequence_lengths).reshape(-1).astype(np.int64)
    nb = lens.shape[0]
    order = np.argsort(-lens, kind="stable")  # descending length
    ngrp = nb // G
    ncores = nb // BLOC
    perm = np.concatenate([
        np.concatenate([order[c * G:(c + 1) * G],
                        order[(ngrp - 1 - c) * G:(ngrp - c) * G]])
        for c in range(ncores)
    ])
    x = np.array(inputs, dtype=np.float32, copy=True)
    tmask = np.arange(t_total)[None, :] >= lens[:, None]  # [B, T]
    x[tmask] = -1e30
    tau = np.arange(t_total + 32)[None, :]
    vmask_full = (tau < lens[:, None]).astype(np.float32)  # [B, T+32]
    in_maps = []
    for c in range(ncores):
        idx = perm[c * BLOC:(c + 1) * BLOC]
        in_maps.append({
            "x": np.ascontiguousarray(x[idx]),
            "trans": np.asarray(transitions, dtype=np.float32),
            "vmask": np.ascontiguousarray(vmask_full[idx]),
            "lb1": (lens[idx] - 1).astype(np.int32)[None, :],
        })
    return in_maps, perm


def kernel(inputs, transitions, sequence_lengths):
    from concourse.bass_utils import run_bass_kernel_spmd
    nc = _get_nc()
    in_maps, perm = make_core_inputs(inputs, transitions, sequence_lengths)
    res = run_bass_kernel_spmd(nc, in_maps, list(range(NCORES)))
    out = np.empty((B, T, C), dtype=np.float32)
    for c in range(NCORES):
        out[perm[c * BLOC:(c + 1) * BLOC]] = res.results[c]["out"]
    return out
